# revision 87
# baseline (speedup 1.0000x reference)
"""Trainium2 Bass kernel for nn_CacaAttention (GQA + RoPE + sliding-window SDPA).

Sharding (8 cores, head tensor-parallel per the hint):
  - core c gets q-heads {2c, 2c+1} (w_q cols), its KV head c//2 (w_k/w_v cols,
    replicated x2 since KV-head groups stay intact), and the matching w_o rows.
  - hidden_states is replicated (projections contract over the full model dim);
    the host pre-transposes it to [HID, S] tile layout so the kernel never
    transposes on the PE.
  - each core emits a partial o_proj output [S, HID] in fp16; the host unshard
    step sums the 8 partials in f32 (the gather for contraction-dim TP).

Everything on-device is fp16 (1.0 PE cycles/row like bf16 but 10 mantissa
bits; rel err ~1e-3 total) with f32 PSUM accumulation, f32 RoPE arithmetic
and f32 softmax statistics. Attention runs in the transposed-score layout
S^T=[k,q]: the softmax denominator comes from a ones-matmul (a partition
reduction, which the PE does faster than DVE/Pool), so no probs transpose is
needed. V is projected directly into natural [t, d] layout by using the hsT
tile as the matmul stationary, so there is no V transpose either.

The whole kernel is one interleaved tile scope — per iteration: project
token-block bi, o_proj q-block bi-1, attend q-block bi (the sliding window
only looks back, so attention needs no tokens beyond its own block). The PE
never drains between phases; PSUM accumulation chains rotate through banks
such that no two pending accumulation groups ever share a bank, and reads
never overlap an active group elsewhere in the same bank (a hardware hazard
CoreSim does not model). RoPE's rotate-half is two batched SBUF-to-SBUF
partition-swap DMAs per token block (the BIR verifier rejects cross-base
partition reads when both tensor-op inputs are in SBUF). Fully-masked
q-half-tiles of the dn/pv chains are skipped by narrowing their APs; the
chain opener is rotated so a full tile always initializes the accumulation
region.
"""
import os
import sys

sys.path.insert(0, "/opt/trn_rl_repo")
import numpy as np

# Problem constants (hardcoded per contract).
B, S, HID = 1, 2048, 2048
NH, NKV, HD = 16, 4, 128
WIN = 1024
THETA = 10000.0
NCORES = 8
HPC = NH // NCORES          # q heads per core
QC = HPC * HD               # q proj cols per core
KC = HID // 128             # contraction chunks
TB = 512                    # projection token block
NTB = S // TB
QB = 256                    # attention query block
NQB = S // QB
NKT = S // 128              # k tiles

_cache = {}


def _rope_tables():
    """cos/sin tables in transposed layout [HD, S]; ssin has the rotate-half
    sign folded in for the partition-offset-add scheme: u = x*ssin, then
    dst[0:64] = t1[0:64] + u[64:128] and dst[64:128] = t1[64:128] + u[0:64],
    so rows 64:128 of ssin carry the minus sign."""
    inv_freq = 1.0 / (THETA ** (np.arange(0, HD, 2, dtype=np.float32) / HD))
    t = np.arange(S, dtype=np.float32)
    freqs = np.outer(t, inv_freq).astype(np.float32)          # [S, HD/2]
    emb = np.concatenate((freqs, freqs), axis=-1)             # [S, HD]
    cos_t = np.cos(emb).T.astype(np.float16).copy()           # [HD, S]
    sin_t = np.sin(emb).T.astype(np.float16).copy()
    ssin = sin_t.copy()
    ssin[HD // 2:] *= -1.0
    return cos_t, ssin


def _mask_bias(delta):
    """Additive bias tile [128(k-part), QB(q-free)]: 0 where
    0 <= (delta + qf - kp) <= WIN else -1e9."""
    kp = np.arange(128)[:, None]
    qf = np.arange(QB)[None, :]
    dist = delta + qf - kp
    bad = (dist < 0) | (dist > WIN)
    return np.where(bad, np.float32(-30000.0), np.float32(0.0)).astype(np.float16)


def _build(niter=1):
    import concourse.bacc as bacc
    import concourse.mybir as mybir
    import concourse.tile as tile

    F32 = mybir.dt.float32
    F16 = mybir.dt.float16

    nc = bacc.Bacc("TRN2", target_bir_lowering=False, debug=False)

    # Host-prepped layouts: hsT is [128, KC, S] (hid on partitions), weights
    # are [128, kc, cols] (contraction chunked on partitions), all fp16.
    # wk/wv are packed into one tensor (512B innermost rows for the DMA).
    hsT = nc.dram_tensor("hsT", [128, KC, S], F16, kind="ExternalInput").ap()
    wq = nc.dram_tensor("wq", [128, KC, QC], F16, kind="ExternalInput").ap()
    wkv = nc.dram_tensor("wkv", [128, KC, 2, HD], F16, kind="ExternalInput").ap()
    wo = nc.dram_tensor("wo", [128, HPC, HID], F16, kind="ExternalInput").ap()
    out = nc.dram_tensor("out", [S, HID], F16, kind="ExternalOutput").ap()

    cos_np, ssin_np = _rope_tables()
    trig_c = nc.inline_tensor(
        np.ascontiguousarray(np.stack([cos_np, ssin_np], axis=1)), "trig_c").ap()
    # pair mask biases [128, 2, QB] (last k-tile-pair causal: delta 0,-128;
    # first pair when q0>=WIN: delta WIN, WIN-128) and the dn ones tile,
    # packed into one inline tensor so startup needs a single DMA.
    causal_np = np.stack([_mask_bias(0), _mask_bias(-128)], axis=1)
    window_np = np.stack([_mask_bias(WIN), _mask_bias(WIN - 128)], axis=1)
    mo_np = np.concatenate([causal_np.reshape(128, 512),
                            window_np.reshape(128, 512),
                            np.ones((128, 128), dtype=np.float16)], axis=1)
    mo_c = nc.inline_tensor(np.ascontiguousarray(mo_np), "mo_c").ap()

    with tile.TileContext(nc) as tc:
        with tc.tile_pool(name="consts", bufs=1) as consts, \
             tc.tile_pool(name="wpool", bufs=1) as wpool, \
             tc.tile_pool(name="persist", bufs=1) as persist:
            trigT = consts.tile([128, 2, S], F16)
            mo = consts.tile([128, 1152], F16)
            cosT = trigT[:, 0, :]
            sinE = trigT[:, 1, :]
            masks = {"causal": mo[:, 0:512], "window": mo[:, 512:1024]}
            ones = mo[:, 1024:1152]

            wq_s = wpool.tile([128, KC, QC], F16)
            wkv_s = wpool.tile([128, KC, 2, HD], F16)
            wo_s = wpool.tile([128, HPC, HID], F16)

            # ---- persistent activations (all fp16) ----
            QT = persist.tile([128, HPC, S], F16)    # roped q, transposed [HD, h, t]
            KT = persist.tile([128, S], F16)         # roped k, transposed
            Vn = persist.tile([128, NKT, HD], F16)   # v natural [t(kt,p), d]
            AT = persist.tile([128, HPC, S], F16)    # attn out, transposed

            consts_aps = dict(trig_c=trig_c, mo_c=mo_c, wq=wq, wkv=wkv, wo=wo)
            for _it in range(niter):
                _body(nc, tc, tile, mybir, F32, F16,
                      hsT, out, wq_s, wkv_s, wo_s,
                      QT, KT, Vn, AT, ones, cosT, sinE, masks, trigT, mo,
                      consts_aps if _it == 0 else None)

    nc.compile()
    return nc


def _body(nc, tc, tile, mybir, F32, F16, hsT, out, wq_s, wkv_s, wo_s,
          QT, KT, Vn, AT, ones, cosT, sinE, masks, trigT, mo, consts_aps=None):
    inv_sqrt_d = 1.0 / float(np.sqrt(HD))
    H2 = HD // 2
    KH = KC // 2

    with tc.tile_pool(name="hTp", bufs=4) as hTp, \
         tc.tile_pool(name="atmp", bufs=2) as atmp, \
         tc.tile_pool(name="epool", bufs=4) as epool, \
         tc.tile_pool(name="rpool", bufs=2) as rpool, \
         tc.tile_pool(name="opool", bufs=6) as opool, \
         tc.tile_pool(name="ps_qk", bufs=2, space="PSUM") as ps_qk, \
         tc.tile_pool(name="ps_s", bufs=2, space="PSUM") as ps_s, \
         tc.tile_pool(name="ps_dp", bufs=2, space="PSUM") as ps_dp, \
         tc.tile_pool(name="ps_o", bufs=2, space="PSUM") as ps_o:

        def project(bi):
            t0 = bi * TB
            hT = hTp.tile([128, KC, TB], F16, tag="hT")
            if consts_aps is not None and bi == 0:
                # Startup-critical ordering: quarter-chunks of hsT-tile0 and
                # the qkv weights land in kc order so the kc=0 matmuls begin
                # ~2us in, then rope tables / masks (needed a bit later),
                # then ones.
                # stream in exact consumption order: q0/q1 chains need only
                # hT+wq; wkv (k/v chains) and masks follow behind
                for qt in range(4):
                    sl = slice(qt * (KC // 4), (qt + 1) * (KC // 4))
                    nc.sync.dma_start(out=hT[:, sl, :], in_=hsT[:, sl, 0:TB])
                    nc.sync.dma_start(out=wq_s[:, sl], in_=consts_aps["wq"][:, sl])
                for qt in range(2):
                    sl = slice(qt * KH, (qt + 1) * KH)
                    nc.sync.dma_start(out=wkv_s[:, sl], in_=consts_aps["wkv"][:, sl])
                nc.sync.dma_start(out=mo, in_=consts_aps["mo_c"])
            else:
                nc.sync.dma_start(out=hT, in_=hsT[:, :, t0:t0 + TB])
            if consts_aps is not None:
                # rope tables arrive as per-TB chunks so they never queue
                # ahead of startup-critical weight traffic
                nc.sync.dma_start(out=trigT[:, :, t0:t0 + TB],
                                  in_=consts_aps["trig_c"][:, :, t0:t0 + TB])
                if bi == 0:
                    nc.sync.dma_start(out=wo_s, in_=consts_aps["wo"])

            # q0 / q1 / k chains rotate through 2 PSUM banks (sequential
            # passes, so never two pending accumulation groups per bank).
            # One broadcast-input mul per chain yields both rope products:
            # ut[:, i, 0, :] = acc*cos, ut[:, i, 1, :] = acc*ssin.
            ut = atmp.tile([128, 3, 2, TB], F16, tag="ut")
            sw = atmp.tile([128, 3, TB], F16, tag="sw")
            for i, stat in enumerate((wq_s[:, :, 0:128], wq_s[:, :, 128:256],
                                      wkv_s[:, :, 0, :])):
                acc_t = ps_qk.tile([128, 2, QB], F32, tag="qk")
                acc = acc_t[:, 0, :]
                for kc in range(KC):
                    nc.tensor.matmul(acc, stat[:, kc, :], hT[:, kc, :],
                                     start=(kc == 0), stop=(kc == KC - 1))
                nc.vector.tensor_mul(ut[:, i, :, :],
                                     acc.unsqueeze(1).broadcast_to((128, 2, TB)),
                                     trigT[:, :, t0:t0 + TB])
            # batched rotate-half swap: two SBUF-to-SBUF DMAs for all 3 ropes
            nc.sync.dma_start(out=sw[0:H2, :, :], in_=ut[H2:128, :, 1, :])
            nc.sync.dma_start(out=sw[H2:128, :, :], in_=ut[0:H2, :, 1, :])

            # v projected straight into natural [t, d] layout (hsT tile as
            # the stationary); two sequential chain passes share one bank,
            # rotating through the same 2-bank tag as the q/k chains.
            vn_t = ps_qk.tile([128, 2, QB], F32, tag="qk")
            vn = vn_t[:, 0, :]
            ntt = TB // 128
            for tt in range(ntt):
                for kc in range(KC):
                    nc.tensor.matmul(vn[:, tt * 128:(tt + 1) * 128],
                                     hT[:, kc, tt * 128:(tt + 1) * 128],
                                     wkv_s[:, kc, 1, :],
                                     start=(kc == 0), stop=(kc == KC - 1))
            nc.vector.tensor_copy(Vn[:, bi * ntt:(bi + 1) * ntt, :], vn)

            nc.vector.tensor_add(QT[:, :, t0:t0 + TB], ut[:, 0:2, 0, :],
                                 sw[:, 0:2, :])
            nc.vector.tensor_add(KT[:, t0:t0 + TB], ut[:, 2, 0, :],
                                 sw[:, 2, :])

        def attend_scores(qb):
            q0 = qb * QB
            kt_lo = max(0, (q0 - WIN) // 128)
            kt_hi = (q0 + QB - 1) // 128
            nkt = kt_hi - kt_lo + 1
            npair = nkt // 2
            # scores+exp for BOTH heads first, then the reduce chains: h=1's
            # score matmuls hide h=0's mask/exp latency on the PE.
            # masked pairs first: their mask-add (DVE) + exp (Act) latency
            # hides under the clean pairs' score matmuls
            if npair == 1:
                pair_order = [0]
            elif q0 >= WIN:
                pair_order = [npair - 1, 0] + list(range(1, npair - 1))
            else:
                pair_order = [npair - 1] + list(range(npair - 1))
            Es = [epool.tile([128, 5, 2, QB], F16, tag="E", name=f"E{h}")
                  for h in range(HPC)]
            nsp = 0
            for pi in pair_order:
                for h in range(HPC):
                    E = Es[h]
                    # final q-block: projections are done, so its score
                    # tiles also rotate through the idle qk banks (deeper
                    # score/exp pipeline right where exp paces the PE)
                    if qb == NQB - 1 and nsp % 2:
                        sp_ = ps_qk.tile([128, 2, QB], F32, tag="qk")
                    else:
                        sp_ = ps_s.tile([128, 2, QB], F32, tag="sp")
                    nsp += 1
                    for j in range(2):
                        kt = kt_lo + pi * 2 + j
                        nc.tensor.matmul(
                            sp_[:, j, :], KT[:, kt * 128:(kt + 1) * 128],
                            QT[:, h, q0:q0 + QB], start=True, stop=True)
                    if pi == npair - 1:
                        nc.vector.tensor_add(sp_, sp_, masks["causal"])
                    elif pi == 0 and q0 >= WIN:
                        nc.vector.tensor_add(sp_, sp_, masks["window"])
                    nc.scalar.activation(
                        E[:, pi, :, :], sp_,
                        mybir.ActivationFunctionType.Exp, scale=inv_sqrt_d)
            return Es

        def attend_reduce(qb, Es):
            q0 = qb * QB
            kt_lo = max(0, (q0 - WIN) // 128)
            kt_hi = (q0 + QB - 1) // 128
            nkt = kt_hi - kt_lo + 1
            for h in range(HPC):
                E = Es[h]
                # dn and pv share one PSUM bank: [:,0,:]=dn, [:,1,:]=pv.
                # pv chain FIRST: the rec/AT-mul reads must not overlap an
                # active accumulation group elsewhere in the bank (hardware
                # hazard CoreSim does not model).
                def ehalf(i):
                    lo, hi = 0, QB
                    if i == nkt - 1:
                        lo = 128
                    elif i == 0 and q0 >= WIN:
                        hi = 128
                    return lo, hi

                # chain order: the opener (start=True) must be a full tile so
                # the whole accumulation region initializes; when the window
                # edge narrows i=0, rotate it to the end of the chain.
                idx = list(range(nkt))
                if q0 >= WIN:
                    idx = idx[1:] + [0]

                dnpv = ps_dp.tile([128, 2, QB], F32, tag="dnpv")
                for row, stat in ((1, None), (0, ones)):
                    for pos, i in enumerate(idx):
                        st, sp__ = (pos == 0), (pos == nkt - 1)
                        lo, hi = ehalf(i)
                        kt = kt_lo + i
                        nc.tensor.matmul(
                            dnpv[:, row, lo:hi],
                            Vn[:, kt, :] if stat is None else stat,
                            E[:, i // 2, i % 2, lo:hi], start=st, stop=sp__)
                rec = rpool.tile([128, QB], F32, tag="rec")
                nc.vector.reciprocal(rec, dnpv[:, 0, :])
                if qb == NQB - 1:
                    # tail: per-token-tile normalize so the last o_proj
                    # chains start half a tile earlier
                    nc.vector.tensor_mul(AT[:, h, q0:q0 + 128],
                                         dnpv[:, 1, 0:128], rec[:, 0:128])
                    nc.vector.tensor_mul(AT[:, h, q0 + 128:q0 + QB],
                                         dnpv[:, 1, 128:QB], rec[:, 128:QB])
                else:
                    nc.vector.tensor_mul(AT[:, h, q0:q0 + QB], dnpv[:, 1, :], rec)

        def oproj(qb, half=None):
            tss = range(qb * (QB // 128), (qb + 1) * (QB // 128))
            if half is not None:
                tss = [list(tss)[half]]
            for ts in tss:
                last = ts == S // 128 - 1
                ost = opool.tile([128, HID], F16, tag="ost")
                for cg in range(HID // 512):
                    if ts >= S // 128 - 2 and cg % 2:
                        opt_ = ps_s.tile([128, 2, QB], F32, tag="sp")
                        op = opt_.rearrange("p a b -> p (a b)")
                    else:
                        op = ps_o.tile([128, 512], F32, tag="op")
                    for ch in range(HPC):
                        nc.tensor.matmul(
                            op, AT[:, ch, ts * 128:(ts + 1) * 128],
                            wo_s[:, ch, cg * 512:(cg + 1) * 512],
                            start=(ch == 0), stop=(ch == HPC - 1))
                    if (ts < S // 128 - 1 and cg != 3) or (ts >= S // 128 - 1 and cg % 2):
                        nc.scalar.copy(ost[:, cg * 512:(cg + 1) * 512], op)
                    else:
                        nc.vector.tensor_copy(ost[:, cg * 512:(cg + 1) * 512], op)
                    if last:
                        # tail: per-column-group DMA so the copy/DMA chain
                        # pipelines instead of waiting for the whole row tile
                        nc.sync.dma_start(
                            out=out[ts * 128:(ts + 1) * 128,
                                    cg * 512:(cg + 1) * 512],
                            in_=ost[:, cg * 512:(cg + 1) * 512])
                if not last:
                    nc.sync.dma_start(
                        out=out[ts * 128:(ts + 1) * 128, :], in_=ost)

        # Soft pipeline: attend(bi) needs only tokens <= its own block
        # (causal window). Per iteration: project(bi), then the previous
        # q-block's o_proj halves bracket the score phase — the first half
        # covers rope(bi)'s DVE latency, the second covers scores(bi)'s
        # mask/exp latency before the reduce phase consumes E.
        qpb = TB // QB
        project(0)
        for q in range(qpb):
            es = attend_scores(q)
            attend_reduce(q, es)
        for bi in range(1, NTB):
            project(bi)
            for q in range(qpb):
                oproj((bi - 1) * qpb + q)
                es = attend_scores(bi * qpb + q)
                attend_reduce(bi * qpb + q, es)
        for q in range(qpb):
            oproj((NTB - 1) * qpb + q)


def _get_nc(niter=1):
    key = f"nc{niter}"
    if key not in _cache:
        _cache[key] = _build(niter)
    return _cache[key]


def _pack_contraction(w):
    """[HID, cols] f32 -> [128, KC, cols] fp16 with hid = kc*128 + p."""
    cols = w.shape[1]
    return np.ascontiguousarray(
        w.reshape(KC, 128, cols).transpose(1, 0, 2).astype(np.float16))


def _shard_inputs(hidden_states, w_q, w_k, w_v, w_o):
    hsf = np.asarray(hidden_states, dtype=np.float32).reshape(S, HID)
    # [S, HID] -> [128, KC, S] fp16 (hid on partitions): hid = kc*128 + p
    hsT = np.ascontiguousarray(
        hsf.T.reshape(KC, 128, S).transpose(1, 0, 2).astype(np.float16))
    w_q = np.asarray(w_q, dtype=np.float32)
    w_k = np.asarray(w_k, dtype=np.float32)
    w_v = np.asarray(w_v, dtype=np.float32)
    w_o = np.asarray(w_o, dtype=np.float32)
    in_maps = []
    for c in range(NCORES):
        kvh = c // (NCORES // NKV)
        wkc = _pack_contraction(w_k[:, kvh * HD:(kvh + 1) * HD])
        wvc = _pack_contraction(w_v[:, kvh * HD:(kvh + 1) * HD])
        wkv = np.ascontiguousarray(
            np.stack([wkc, wvc], axis=2))        # [128, KC, 2, HD]
        # wo shard [QC, HID] -> [128, HPC, HID]: row = ch*128 + p
        woc = w_o[c * QC:(c + 1) * QC, :]
        woc = np.ascontiguousarray(
            woc.reshape(HPC, 128, HID).transpose(1, 0, 2).astype(np.float16))
        in_maps.append({
            "hsT": hsT,
            "wq": _pack_contraction(w_q[:, c * QC:(c + 1) * QC]),
            "wkv": wkv,
            "wo": woc,
        })
    return in_maps


def _get_runner(niter=1):
    """Jitted 8-core executor with device-resident zero-out buffers (no
    donation, so repeated timed calls don't re-upload)."""
    rkey = ("runner", niter)
    if rkey in _cache:
        return _cache[rkey]
    import jax
    import concourse.mybir as mybir
    from jax.sharding import Mesh, PartitionSpec
    from jax.experimental.shard_map import shard_map
    from concourse.bass2jax import (
        _bass_exec_p, install_neuronx_cc_hook, partition_id_tensor)

    install_neuronx_cc_hook()
    nc = _get_nc(niter)
    pname = nc.partition_id_tensor.name if nc.partition_id_tensor else None

    in_names, out_names, out_avals = [], [], []
    for alloc in nc.m.functions[0].allocations:
        if not isinstance(alloc, mybir.MemoryLocationSet):
            continue
        name = alloc.memorylocations[0].name
        if alloc.kind == "ExternalInput":
            if name != pname:
                in_names.append(name)
        elif alloc.kind == "ExternalOutput":
            out_names.append(name)
            out_avals.append(jax.core.ShapedArray(
                tuple(alloc.tensor_shape), mybir.dt.np(alloc.dtype)))
    n_params = len(in_names)
    all_names = in_names + out_names
    if pname is not None:
        all_names = all_names + [pname]

    def _body(*args):
        operands = list(args)
        if pname is not None:
            operands.append(partition_id_tensor())
        outs = _bass_exec_p.bind(
            *operands,
            out_avals=tuple(out_avals),
            in_names=tuple(all_names),
            out_names=tuple(out_names),
            lowering_input_output_aliases=(),
            sim_require_finite=True,
            sim_require_nnan=True,
            nc=nc,
        )
        return tuple(outs)

    devices = jax.devices()[:NCORES]
    mesh = Mesh(np.asarray(devices), ("core",))
    nspec = n_params + len(out_names)
    fn = jax.jit(shard_map(
        _body, mesh=mesh,
        in_specs=(PartitionSpec("core"),) * nspec,
        out_specs=(PartitionSpec("core"),) * len(out_names),
        check_rep=False))
    _cache[rkey] = (fn, in_names, out_names, out_avals)
    return _cache[rkey]


def _prep_device_args(in_maps):
    import jax
    fn, in_names, out_names, out_avals = _get_runner()
    concat_in = [np.concatenate([np.asarray(in_maps[c][n]) for c in range(NCORES)], axis=0)
                 for n in in_names]
    zeros = [np.zeros((NCORES * a.shape[0], *a.shape[1:]), a.dtype) for a in out_avals]
    return [jax.device_put(x) for x in concat_in + zeros]


def _run(in_maps):
    fn, in_names, out_names, out_avals = _get_runner()
    args = _prep_device_args(in_maps)
    outs = fn(*args)
    _cache["last_args"] = args
    return [
        {n: np.asarray(outs[i]).reshape(NCORES, *out_avals[i].shape)[c]
         for i, n in enumerate(out_names)}
        for c in range(NCORES)
    ]


def time_kernel(reps=10, n=16, m=16):
    """Marginal per-kernel-iteration device time (ns): pipelined loops of m
    dispatches of an n-iteration-unrolled build vs the 1-iteration build.
    Dispatch overhead (~31ms/call, pipelined) cancels in the difference.
    Noisy on this axon setup — treat as a rough cross-check of the
    cost-model (TimelineSim) estimate."""
    import time
    args = _cache.get("last_args")
    assert args is not None, "run kernel() first"

    def timed(niter):
        fn, _, _, _ = _get_runner(niter)
        for o in fn(*args):
            o.block_until_ready()  # warm/compile
        ts = []
        for _ in range(reps):
            t0 = time.perf_counter()
            outs = None
            for _ in range(m):
                outs = fn(*args)
            for o in outs:
                o.block_until_ready()
            ts.append((time.perf_counter() - t0) / m)
        return ts

    t1 = sorted(timed(1))
    tn = sorted(timed(n))
    print(f"  niter=1 : " + " ".join(f"{t*1e3:.2f}" for t in t1), flush=True)
    print(f"  niter={n}: " + " ".join(f"{t*1e3:.2f}" for t in tn), flush=True)
    k = max(2, reps // 3)
    est = (sum(tn[:k]) / k - sum(t1[:k]) / k) / (n - 1) * 1e9
    return est


def kernel(hidden_states, w_q, w_k, w_v, w_o):
    in_maps = _shard_inputs(hidden_states, w_q, w_k, w_v, w_o)
    results = _run(in_maps)
    acc = np.zeros((S, HID), dtype=np.float32)
    for c in range(NCORES):
        acc += results[c]["out"].astype(np.float32)
    return acc.reshape(B, S, HID)


# revision 92
# speedup vs baseline: 1.0027x; 1.0027x over previous
"""Trainium2 Bass kernel for nn_CacaAttention (GQA + RoPE + sliding-window SDPA).

Sharding (8 cores, head tensor-parallel per the hint):
  - core c gets q-heads {2c, 2c+1} (w_q cols), its KV head c//2 (w_k/w_v cols,
    replicated x2 since KV-head groups stay intact), and the matching w_o rows.
  - hidden_states is replicated (projections contract over the full model dim);
    the host pre-transposes it to [HID, S] tile layout so the kernel never
    transposes on the PE.
  - each core emits a partial o_proj output [S, HID] in fp16; the host unshard
    step sums the 8 partials in f32 (the gather for contraction-dim TP).

Everything on-device is fp16 (1.0 PE cycles/row like bf16 but 10 mantissa
bits; rel err ~1e-3 total) with f32 PSUM accumulation, f32 RoPE arithmetic
and f32 softmax statistics. Attention runs in the transposed-score layout
S^T=[k,q]: the softmax denominator comes from a ones-matmul (a partition
reduction, which the PE does faster than DVE/Pool), so no probs transpose is
needed. V is projected directly into natural [t, d] layout by using the hsT
tile as the matmul stationary, so there is no V transpose either.

The whole kernel is one interleaved tile scope — per iteration: project
token-block bi, o_proj q-block bi-1, attend q-block bi (the sliding window
only looks back, so attention needs no tokens beyond its own block). The PE
never drains between phases; PSUM accumulation chains rotate through banks
such that no two pending accumulation groups ever share a bank, and reads
never overlap an active group elsewhere in the same bank (a hardware hazard
CoreSim does not model). RoPE's rotate-half is two batched SBUF-to-SBUF
partition-swap DMAs per token block (the BIR verifier rejects cross-base
partition reads when both tensor-op inputs are in SBUF). Fully-masked
q-half-tiles of the dn/pv chains are skipped by narrowing their APs; the
chain opener is rotated so a full tile always initializes the accumulation
region.
"""
import os
import sys

sys.path.insert(0, "/opt/trn_rl_repo")
import numpy as np

# Problem constants (hardcoded per contract).
B, S, HID = 1, 2048, 2048
NH, NKV, HD = 16, 4, 128
WIN = 1024
THETA = 10000.0
NCORES = 8
HPC = NH // NCORES          # q heads per core
QC = HPC * HD               # q proj cols per core
KC = HID // 128             # contraction chunks
TB = 512                    # projection token block
NTB = S // TB
QB = 256                    # attention query block
NQB = S // QB
NKT = S // 128              # k tiles

_cache = {}


def _rope_tables():
    """cos/sin tables in transposed layout [HD, S]; ssin has the rotate-half
    sign folded in for the partition-offset-add scheme: u = x*ssin, then
    dst[0:64] = t1[0:64] + u[64:128] and dst[64:128] = t1[64:128] + u[0:64],
    so rows 64:128 of ssin carry the minus sign."""
    inv_freq = 1.0 / (THETA ** (np.arange(0, HD, 2, dtype=np.float32) / HD))
    t = np.arange(S, dtype=np.float32)
    freqs = np.outer(t, inv_freq).astype(np.float32)          # [S, HD/2]
    emb = np.concatenate((freqs, freqs), axis=-1)             # [S, HD]
    cos_t = np.cos(emb).T.astype(np.float16).copy()           # [HD, S]
    sin_t = np.sin(emb).T.astype(np.float16).copy()
    ssin = sin_t.copy()
    ssin[HD // 2:] *= -1.0
    return cos_t, ssin


def _mask_bias(delta):
    """Additive bias tile [128(k-part), QB(q-free)]: 0 where
    0 <= (delta + qf - kp) <= WIN else -1e9."""
    kp = np.arange(128)[:, None]
    qf = np.arange(QB)[None, :]
    dist = delta + qf - kp
    bad = (dist < 0) | (dist > WIN)
    return np.where(bad, np.float32(-30000.0), np.float32(0.0)).astype(np.float16)


def _build(niter=1):
    import concourse.bacc as bacc
    import concourse.mybir as mybir
    import concourse.tile as tile

    F32 = mybir.dt.float32
    F16 = mybir.dt.float16

    nc = bacc.Bacc("TRN2", target_bir_lowering=False, debug=False)

    # Host-prepped layouts: hsT is [128, KC, S] (hid on partitions), weights
    # are [128, kc, cols] (contraction chunked on partitions), all fp16.
    # wk/wv are packed into one tensor (512B innermost rows for the DMA).
    hsT = nc.dram_tensor("hsT", [128, KC, S], F16, kind="ExternalInput").ap()
    wq = nc.dram_tensor("wq", [128, KC, QC], F16, kind="ExternalInput").ap()
    wkv = nc.dram_tensor("wkv", [128, KC, 2, HD], F16, kind="ExternalInput").ap()
    wo = nc.dram_tensor("wo", [128, HPC, HID], F16, kind="ExternalInput").ap()
    out = nc.dram_tensor("out", [S, HID], F16, kind="ExternalOutput").ap()

    cos_np, ssin_np = _rope_tables()
    trig_c = nc.inline_tensor(
        np.ascontiguousarray(np.stack([cos_np, ssin_np], axis=1)), "trig_c").ap()
    # pair mask biases [128, 2, QB] (last k-tile-pair causal: delta 0,-128;
    # first pair when q0>=WIN: delta WIN, WIN-128) and the dn ones tile,
    # packed into one inline tensor so startup needs a single DMA.
    causal_np = np.stack([_mask_bias(0), _mask_bias(-128)], axis=1)
    window_np = np.stack([_mask_bias(WIN), _mask_bias(WIN - 128)], axis=1)
    mo_np = np.concatenate([causal_np.reshape(128, 512),
                            window_np.reshape(128, 512),
                            np.ones((128, 128), dtype=np.float16)], axis=1)
    mo_c = nc.inline_tensor(np.ascontiguousarray(mo_np), "mo_c").ap()

    with tile.TileContext(nc) as tc:
        with tc.tile_pool(name="consts", bufs=1) as consts, \
             tc.tile_pool(name="wpool", bufs=1) as wpool, \
             tc.tile_pool(name="persist", bufs=1) as persist:
            trigT = consts.tile([128, 2, S], F16)
            mo = consts.tile([128, 1152], F16)
            cosT = trigT[:, 0, :]
            sinE = trigT[:, 1, :]
            masks = {"causal": mo[:, 0:512], "window": mo[:, 512:1024]}
            ones = mo[:, 1024:1152]

            wq_s = wpool.tile([128, KC, QC], F16)
            wkv_s = wpool.tile([128, KC, 2, HD], F16)
            wo_s = wpool.tile([128, HPC, HID], F16)

            # ---- persistent activations (all fp16) ----
            QT = persist.tile([128, HPC, S], F16)    # roped q, transposed [HD, h, t]
            KT = persist.tile([128, S], F16)         # roped k, transposed
            Vn = persist.tile([128, NKT, HD], F16)   # v natural [t(kt,p), d]
            AT = persist.tile([128, HPC, S], F16)    # attn out, transposed

            consts_aps = dict(trig_c=trig_c, mo_c=mo_c, wq=wq, wkv=wkv, wo=wo)
            for _it in range(niter):
                _body(nc, tc, tile, mybir, F32, F16,
                      hsT, out, wq_s, wkv_s, wo_s,
                      QT, KT, Vn, AT, ones, cosT, sinE, masks, trigT, mo,
                      consts_aps if _it == 0 else None)

    nc.compile()
    return nc


def _body(nc, tc, tile, mybir, F32, F16, hsT, out, wq_s, wkv_s, wo_s,
          QT, KT, Vn, AT, ones, cosT, sinE, masks, trigT, mo, consts_aps=None):
    inv_sqrt_d = 1.0 / float(np.sqrt(HD))
    H2 = HD // 2
    KH = KC // 2

    with tc.tile_pool(name="hTp", bufs=4) as hTp, \
         tc.tile_pool(name="atmp", bufs=2) as atmp, \
         tc.tile_pool(name="epool", bufs=4) as epool, \
         tc.tile_pool(name="rpool", bufs=2) as rpool, \
         tc.tile_pool(name="opool", bufs=6) as opool, \
         tc.tile_pool(name="ps_qk", bufs=2, space="PSUM") as ps_qk, \
         tc.tile_pool(name="ps_s", bufs=2, space="PSUM") as ps_s, \
         tc.tile_pool(name="ps_dp", bufs=2, space="PSUM") as ps_dp, \
         tc.tile_pool(name="ps_o", bufs=2, space="PSUM") as ps_o:

        def project(bi):
            t0 = bi * TB
            hT = hTp.tile([128, KC, TB], F16, tag="hT")
            if consts_aps is not None and bi == 0:
                # Startup-critical ordering: quarter-chunks of hsT-tile0 and
                # the qkv weights land in kc order so the kc=0 matmuls begin
                # ~2us in, then rope tables / masks (needed a bit later),
                # then ones.
                # stream in exact consumption order: q0/q1 chains need only
                # hT+wq; wkv (k/v chains) and masks follow behind
                for qt in range(4):
                    sl = slice(qt * (KC // 4), (qt + 1) * (KC // 4))
                    nc.sync.dma_start(out=hT[:, sl, :], in_=hsT[:, sl, 0:TB])
                    nc.sync.dma_start(out=wq_s[:, sl], in_=consts_aps["wq"][:, sl])
                for qt in range(2):
                    sl = slice(qt * KH, (qt + 1) * KH)
                    nc.sync.dma_start(out=wkv_s[:, sl], in_=consts_aps["wkv"][:, sl])
                nc.sync.dma_start(out=mo, in_=consts_aps["mo_c"])
            else:
                nc.sync.dma_start(out=hT, in_=hsT[:, :, t0:t0 + TB])
            if consts_aps is not None:
                # rope tables arrive as per-TB chunks so they never queue
                # ahead of startup-critical weight traffic
                nc.sync.dma_start(out=trigT[:, :, t0:t0 + TB],
                                  in_=consts_aps["trig_c"][:, :, t0:t0 + TB])
                if bi == 0:
                    nc.sync.dma_start(out=wo_s, in_=consts_aps["wo"])

            # q0 / q1 / k chains rotate through 2 PSUM banks (sequential
            # passes, so never two pending accumulation groups per bank).
            # One broadcast-input mul per chain yields both rope products:
            # ut[:, i, 0, :] = acc*cos, ut[:, i, 1, :] = acc*ssin.
            ut = atmp.tile([128, 3, 2, TB], F16, tag="ut")
            sw = atmp.tile([128, 3, TB], F16, tag="sw")
            for i, stat in enumerate((wq_s[:, :, 0:128], wq_s[:, :, 128:256],
                                      wkv_s[:, :, 0, :])):
                acc_t = ps_qk.tile([128, 2, QB], F32, tag="qk")
                acc = acc_t[:, 0, :]
                for kc in range(KC):
                    nc.tensor.matmul(acc, stat[:, kc, :], hT[:, kc, :],
                                     start=(kc == 0), stop=(kc == KC - 1))
                nc.vector.tensor_mul(ut[:, i, :, :],
                                     acc.unsqueeze(1).broadcast_to((128, 2, TB)),
                                     trigT[:, :, t0:t0 + TB])
            # batched rotate-half swap: two SBUF-to-SBUF DMAs for all 3 ropes
            nc.sync.dma_start(out=sw[0:H2, :, :], in_=ut[H2:128, :, 1, :])
            nc.sync.dma_start(out=sw[H2:128, :, :], in_=ut[0:H2, :, 1, :])

            # v projected straight into natural [t, d] layout (hsT tile as
            # the stationary); two sequential chain passes share one bank,
            # rotating through the same 2-bank tag as the q/k chains.
            vn_t = ps_qk.tile([128, 2, QB], F32, tag="qk")
            vn = vn_t[:, 0, :]
            ntt = TB // 128
            for tt in range(ntt):
                for kc in range(KC):
                    nc.tensor.matmul(vn[:, tt * 128:(tt + 1) * 128],
                                     hT[:, kc, tt * 128:(tt + 1) * 128],
                                     wkv_s[:, kc, 1, :],
                                     start=(kc == 0), stop=(kc == KC - 1))
            nc.vector.tensor_copy(Vn[:, bi * ntt:(bi + 1) * ntt, :], vn)

            nc.vector.tensor_add(QT[:, :, t0:t0 + TB], ut[:, 0:2, 0, :],
                                 sw[:, 0:2, :])
            nc.vector.tensor_add(KT[:, t0:t0 + TB], ut[:, 2, 0, :],
                                 sw[:, 2, :])

        def attend_scores(qb):
            q0 = qb * QB
            kt_lo = max(0, (q0 - WIN) // 128)
            kt_hi = (q0 + QB - 1) // 128
            nkt = kt_hi - kt_lo + 1
            npair = nkt // 2
            # scores+exp for BOTH heads first, then the reduce chains: h=1's
            # score matmuls hide h=0's mask/exp latency on the PE.
            # masked pairs first: their mask-add (DVE) + exp (Act) latency
            # hides under the clean pairs' score matmuls
            if npair == 1:
                pair_order = [0]
            elif q0 >= WIN:
                pair_order = [npair - 1, 0] + list(range(1, npair - 1))
            else:
                pair_order = [npair - 1] + list(range(npair - 1))
            Es = [epool.tile([128, 5, 2, QB], F16, tag="E", name=f"E{h}")
                  for h in range(HPC)]
            nsp = 0
            for pi in pair_order:
                for h in range(HPC):
                    E = Es[h]
                    # final q-block: projections are done, so its score
                    # tiles also rotate through the idle qk banks (deeper
                    # score/exp pipeline right where exp paces the PE)
                    if qb == NQB - 1 and (nsp + 1) % 2:
                        sp_ = ps_qk.tile([128, 2, QB], F32, tag="qk")
                    else:
                        sp_ = ps_s.tile([128, 2, QB], F32, tag="sp")
                    nsp += 1
                    for j in range(2):
                        kt = kt_lo + pi * 2 + j
                        nc.tensor.matmul(
                            sp_[:, j, :], KT[:, kt * 128:(kt + 1) * 128],
                            QT[:, h, q0:q0 + QB], start=True, stop=True)
                    if pi == npair - 1:
                        nc.vector.tensor_add(sp_, sp_, masks["causal"])
                    elif pi == 0 and q0 >= WIN:
                        nc.vector.tensor_add(sp_, sp_, masks["window"])
                    nc.scalar.activation(
                        E[:, pi, :, :], sp_,
                        mybir.ActivationFunctionType.Exp, scale=inv_sqrt_d)
            return Es

        def attend_reduce(qb, Es):
            q0 = qb * QB
            kt_lo = max(0, (q0 - WIN) // 128)
            kt_hi = (q0 + QB - 1) // 128
            nkt = kt_hi - kt_lo + 1
            for h in range(HPC):
                E = Es[h]
                # dn and pv share one PSUM bank: [:,0,:]=dn, [:,1,:]=pv.
                # pv chain FIRST: the rec/AT-mul reads must not overlap an
                # active accumulation group elsewhere in the bank (hardware
                # hazard CoreSim does not model).
                def ehalf(i):
                    lo, hi = 0, QB
                    if i == nkt - 1:
                        lo = 128
                    elif i == 0 and q0 >= WIN:
                        hi = 128
                    return lo, hi

                # chain order: the opener (start=True) must be a full tile so
                # the whole accumulation region initializes; when the window
                # edge narrows i=0, rotate it to the end of the chain.
                idx = list(range(nkt))
                if q0 >= WIN:
                    idx = idx[1:] + [0]

                dnpv = ps_dp.tile([128, 2, QB], F32, tag="dnpv")
                for row, stat in ((1, None), (0, ones)):
                    for pos, i in enumerate(idx):
                        st, sp__ = (pos == 0), (pos == nkt - 1)
                        lo, hi = ehalf(i)
                        kt = kt_lo + i
                        nc.tensor.matmul(
                            dnpv[:, row, lo:hi],
                            Vn[:, kt, :] if stat is None else stat,
                            E[:, i // 2, i % 2, lo:hi], start=st, stop=sp__)
                rec = rpool.tile([128, QB], F32, tag="rec")
                nc.vector.reciprocal(rec, dnpv[:, 0, :])
                if qb == NQB - 1:
                    # tail: per-token-tile normalize so the last o_proj
                    # chains start half a tile earlier
                    nc.vector.tensor_mul(AT[:, h, q0:q0 + 128],
                                         dnpv[:, 1, 0:128], rec[:, 0:128])
                    nc.vector.tensor_mul(AT[:, h, q0 + 128:q0 + QB],
                                         dnpv[:, 1, 128:QB], rec[:, 128:QB])
                else:
                    nc.vector.tensor_mul(AT[:, h, q0:q0 + QB], dnpv[:, 1, :], rec)

        def oproj(qb, half=None):
            tss = range(qb * (QB // 128), (qb + 1) * (QB // 128))
            if half is not None:
                tss = [list(tss)[half]]
            for ts in tss:
                last = ts == S // 128 - 1
                ost = opool.tile([128, HID], F16, tag="ost")
                for cg in range(HID // 512):
                    if ts >= S // 128 - 2 and cg % 2:
                        opt_ = ps_s.tile([128, 2, QB], F32, tag="sp")
                        op = opt_.rearrange("p a b -> p (a b)")
                    else:
                        op = ps_o.tile([128, 512], F32, tag="op")
                    for ch in range(HPC):
                        nc.tensor.matmul(
                            op, AT[:, ch, ts * 128:(ts + 1) * 128],
                            wo_s[:, ch, cg * 512:(cg + 1) * 512],
                            start=(ch == 0), stop=(ch == HPC - 1))
                    if (ts < S // 128 - 1 and cg != 3) or (ts >= S // 128 - 1 and cg % 2):
                        nc.scalar.copy(ost[:, cg * 512:(cg + 1) * 512], op)
                    else:
                        nc.vector.tensor_copy(ost[:, cg * 512:(cg + 1) * 512], op)
                    if last:
                        # tail: per-column-group DMA so the copy/DMA chain
                        # pipelines instead of waiting for the whole row tile
                        nc.sync.dma_start(
                            out=out[ts * 128:(ts + 1) * 128,
                                    cg * 512:(cg + 1) * 512],
                            in_=ost[:, cg * 512:(cg + 1) * 512])
                if not last:
                    nc.sync.dma_start(
                        out=out[ts * 128:(ts + 1) * 128, :], in_=ost)

        # Soft pipeline: attend(bi) needs only tokens <= its own block
        # (causal window). Per iteration: project(bi), then the previous
        # q-block's o_proj halves bracket the score phase — the first half
        # covers rope(bi)'s DVE latency, the second covers scores(bi)'s
        # mask/exp latency before the reduce phase consumes E.
        qpb = TB // QB
        project(0)
        for q in range(qpb):
            es = attend_scores(q)
            attend_reduce(q, es)
        for bi in range(1, NTB):
            project(bi)
            for q in range(qpb):
                oproj((bi - 1) * qpb + q)
                es = attend_scores(bi * qpb + q)
                attend_reduce(bi * qpb + q, es)
        for q in range(qpb):
            oproj((NTB - 1) * qpb + q)


def _get_nc(niter=1):
    key = f"nc{niter}"
    if key not in _cache:
        _cache[key] = _build(niter)
    return _cache[key]


def _pack_contraction(w):
    """[HID, cols] f32 -> [128, KC, cols] fp16 with hid = kc*128 + p."""
    cols = w.shape[1]
    return np.ascontiguousarray(
        w.reshape(KC, 128, cols).transpose(1, 0, 2).astype(np.float16))


def _shard_inputs(hidden_states, w_q, w_k, w_v, w_o):
    hsf = np.asarray(hidden_states, dtype=np.float32).reshape(S, HID)
    # [S, HID] -> [128, KC, S] fp16 (hid on partitions): hid = kc*128 + p
    hsT = np.ascontiguousarray(
        hsf.T.reshape(KC, 128, S).transpose(1, 0, 2).astype(np.float16))
    w_q = np.asarray(w_q, dtype=np.float32)
    w_k = np.asarray(w_k, dtype=np.float32)
    w_v = np.asarray(w_v, dtype=np.float32)
    w_o = np.asarray(w_o, dtype=np.float32)
    in_maps = []
    for c in range(NCORES):
        kvh = c // (NCORES // NKV)
        wkc = _pack_contraction(w_k[:, kvh * HD:(kvh + 1) * HD])
        wvc = _pack_contraction(w_v[:, kvh * HD:(kvh + 1) * HD])
        wkv = np.ascontiguousarray(
            np.stack([wkc, wvc], axis=2))        # [128, KC, 2, HD]
        # wo shard [QC, HID] -> [128, HPC, HID]: row = ch*128 + p
        woc = w_o[c * QC:(c + 1) * QC, :]
        woc = np.ascontiguousarray(
            woc.reshape(HPC, 128, HID).transpose(1, 0, 2).astype(np.float16))
        in_maps.append({
            "hsT": hsT,
            "wq": _pack_contraction(w_q[:, c * QC:(c + 1) * QC]),
            "wkv": wkv,
            "wo": woc,
        })
    return in_maps


def _get_runner(niter=1):
    """Jitted 8-core executor with device-resident zero-out buffers (no
    donation, so repeated timed calls don't re-upload)."""
    rkey = ("runner", niter)
    if rkey in _cache:
        return _cache[rkey]
    import jax
    import concourse.mybir as mybir
    from jax.sharding import Mesh, PartitionSpec
    from jax.experimental.shard_map import shard_map
    from concourse.bass2jax import (
        _bass_exec_p, install_neuronx_cc_hook, partition_id_tensor)

    install_neuronx_cc_hook()
    nc = _get_nc(niter)
    pname = nc.partition_id_tensor.name if nc.partition_id_tensor else None

    in_names, out_names, out_avals = [], [], []
    for alloc in nc.m.functions[0].allocations:
        if not isinstance(alloc, mybir.MemoryLocationSet):
            continue
        name = alloc.memorylocations[0].name
        if alloc.kind == "ExternalInput":
            if name != pname:
                in_names.append(name)
        elif alloc.kind == "ExternalOutput":
            out_names.append(name)
            out_avals.append(jax.core.ShapedArray(
                tuple(alloc.tensor_shape), mybir.dt.np(alloc.dtype)))
    n_params = len(in_names)
    all_names = in_names + out_names
    if pname is not None:
        all_names = all_names + [pname]

    def _body(*args):
        operands = list(args)
        if pname is not None:
            operands.append(partition_id_tensor())
        outs = _bass_exec_p.bind(
            *operands,
            out_avals=tuple(out_avals),
            in_names=tuple(all_names),
            out_names=tuple(out_names),
            lowering_input_output_aliases=(),
            sim_require_finite=True,
            sim_require_nnan=True,
            nc=nc,
        )
        return tuple(outs)

    devices = jax.devices()[:NCORES]
    mesh = Mesh(np.asarray(devices), ("core",))
    nspec = n_params + len(out_names)
    fn = jax.jit(shard_map(
        _body, mesh=mesh,
        in_specs=(PartitionSpec("core"),) * nspec,
        out_specs=(PartitionSpec("core"),) * len(out_names),
        check_rep=False))
    _cache[rkey] = (fn, in_names, out_names, out_avals)
    return _cache[rkey]


def _prep_device_args(in_maps):
    import jax
    fn, in_names, out_names, out_avals = _get_runner()
    concat_in = [np.concatenate([np.asarray(in_maps[c][n]) for c in range(NCORES)], axis=0)
                 for n in in_names]
    zeros = [np.zeros((NCORES * a.shape[0], *a.shape[1:]), a.dtype) for a in out_avals]
    return [jax.device_put(x) for x in concat_in + zeros]


def _run(in_maps):
    fn, in_names, out_names, out_avals = _get_runner()
    args = _prep_device_args(in_maps)
    outs = fn(*args)
    _cache["last_args"] = args
    return [
        {n: np.asarray(outs[i]).reshape(NCORES, *out_avals[i].shape)[c]
         for i, n in enumerate(out_names)}
        for c in range(NCORES)
    ]


def time_kernel(reps=10, n=16, m=16):
    """Marginal per-kernel-iteration device time (ns): pipelined loops of m
    dispatches of an n-iteration-unrolled build vs the 1-iteration build.
    Dispatch overhead (~31ms/call, pipelined) cancels in the difference.
    Noisy on this axon setup — treat as a rough cross-check of the
    cost-model (TimelineSim) estimate."""
    import time
    args = _cache.get("last_args")
    assert args is not None, "run kernel() first"

    def timed(niter):
        fn, _, _, _ = _get_runner(niter)
        for o in fn(*args):
            o.block_until_ready()  # warm/compile
        ts = []
        for _ in range(reps):
            t0 = time.perf_counter()
            outs = None
            for _ in range(m):
                outs = fn(*args)
            for o in outs:
                o.block_until_ready()
            ts.append((time.perf_counter() - t0) / m)
        return ts

    t1 = sorted(timed(1))
    tn = sorted(timed(n))
    print(f"  niter=1 : " + " ".join(f"{t*1e3:.2f}" for t in t1), flush=True)
    print(f"  niter={n}: " + " ".join(f"{t*1e3:.2f}" for t in tn), flush=True)
    k = max(2, reps // 3)
    est = (sum(tn[:k]) / k - sum(t1[:k]) / k) / (n - 1) * 1e9
    return est


def kernel(hidden_states, w_q, w_k, w_v, w_o):
    in_maps = _shard_inputs(hidden_states, w_q, w_k, w_v, w_o)
    results = _run(in_maps)
    acc = np.zeros((S, HID), dtype=np.float32)
    for c in range(NCORES):
        acc += results[c]["out"].astype(np.float32)
    return acc.reshape(B, S, HID)
